# revision 27
# baseline (speedup 1.0000x reference)
"""BertSelfAttention (with segment-embedding score bias) on 8 trn2 NeuronCores.

Math implemented (reference semantics):
    q = X @ Wq.T + bq ; k = X @ Wk.T ; v = X @ Wv.T + bv      (per head h)
    scores = (q*s) @ k.T + (q + b_q_s) @ segrep.T + mask ;  s = 1/sqrt(DH)
    out = softmax(scores) @ v

Formulation: per head, augmented 128-deep contractions
    qhat = [q*s ; q + b_q_s]   (dims 0:64 scaled, 64:128 plain+bias)
    khat = [k   ; segrep     ] (segrep = seg_table[seg_ids] slice, host-prep)
    scores = qhat . khat  (exactly includes the segment term); mask is a
    per-key bias fused into the exp() activation. The K=128 contraction keeps
    the PE array fully occupied (half-height matmuls starve the activity
    monitor and the PE clock throttles to 1.2 GHz).
    Softmax denominator = ones-column appended to V in the PV matmul
    (ctx^T accumulated with V stationary, then transposed back per 128-query
    tile and scaled by the reciprocal denominator).

Sharding: tensor-parallel over heads; core c owns heads 2c, 2c+1.
Each core reads the full tokens, computes its head-slice and its slice of
the output; host concatenates along the hidden dim. No collectives.

Schedule: batches processed end-to-end; attention software-pipelined so PV
of iteration g-1 interleaves with scores of iteration g (PE densely busy
while ACT exp()s run concurrently).
"""

import os
import sys

for _p in ("/opt/trn_rl_repo", "/root/.axon_site/_ro/trn_rl_repo"):
    if os.path.isdir(_p) and _p not in sys.path:
        sys.path.append(_p)

import numpy as np
import ml_dtypes

B, S, NH, DH = 4, 2048, 16, 64
HID = NH * DH          # 1024
T = B * S              # 8192
N_CORES = 8
HPC = NH // N_CORES    # heads per core = 2
DPC = HPC * DH         # out dims per core = 128
SCALE = 1.0 / 8.0      # 1/sqrt(DH)
KT = HID // 128        # 8 contraction tiles
CHUNK = 1024           # token chunk for projections
SKT = S // 128         # 16 key tiles per sequence
QH = 2                 # query halves per sequence
QBLK = S // QH         # 1024
NQT = QBLK // 128      # 8 query tiles per half

_cache = {}


def _build_program():
    import concourse.bacc as bacc
    import concourse.tile as tile
    from concourse import masks, mybir
    from contextlib import ExitStack

    bf16 = mybir.dt.bfloat16
    f32 = mybir.dt.float32
    Exp = mybir.ActivationFunctionType.Exp
    Ident = mybir.ActivationFunctionType.Identity

    nc = bacc.Bacc("TRN2", target_bir_lowering=False, debug=False,
                   num_devices=N_CORES)
    xb = nc.dram_tensor("xb", [HID, T], bf16, kind="ExternalInput")
    wq = nc.dram_tensor("wq", [HID, DPC], bf16, kind="ExternalInput")
    wk = nc.dram_tensor("wk", [HID, DPC], bf16, kind="ExternalInput")
    wv = nc.dram_tensor("wv", [HID, DPC], bf16, kind="ExternalInput")
    srt = nc.dram_tensor("srt", [128, T], bf16, kind="ExternalInput")
    rb = nc.dram_tensor("rb", [128, 128], f32, kind="ExternalInput")
    bqa = nc.dram_tensor("bqa", [DPC, 1], f32, kind="ExternalInput")
    bqb = nc.dram_tensor("bqb", [DPC, 1], f32, kind="ExternalInput")
    bv = nc.dram_tensor("bv", [DPC, 1], f32, kind="ExternalInput")
    outd = nc.dram_tensor("out", [T, DPC], f32, kind="ExternalOutput")
    # cross-partition bounce for the plain-q half of qhat
    qbounce = nc.dram_tensor("qbounce", [2, T // CHUNK, 64, CHUNK], bf16)

    with tile.TileContext(nc) as tc, ExitStack() as octx:
        const = octx.enter_context(tc.tile_pool(name="const", bufs=1))
        res = octx.enter_context(tc.tile_pool(name="res", bufs=1))
        xt_pool = octx.enter_context(tc.tile_pool(name="xt", bufs=20))
        vt_pool = octx.enter_context(tc.tile_pool(name="vt", bufs=2))
        pt_pool = octx.enter_context(tc.tile_pool(name="pt", bufs=18))
        ctxs_pool = octx.enter_context(tc.tile_pool(name="ctxs", bufs=2))
        qstage_pool = octx.enter_context(tc.tile_pool(name="qstage", bufs=2))
        stage_pool = octx.enter_context(tc.tile_pool(name="stage", bufs=2))
        rcp_pool = octx.enter_context(tc.tile_pool(name="rcp", bufs=8))
        big_psum = octx.enter_context(
            tc.tile_pool(name="bigp", bufs=2, space="PSUM"))
        ctx_psum = octx.enter_context(
            tc.tile_pool(name="ctxp", bufs=1, space="PSUM"))
        small_psum = octx.enter_context(
            tc.tile_pool(name="smallp", bufs=2, space="PSUM"))

        # constants
        rb_sb = const.tile([128, 128], f32)
        bqa_sb = const.tile([DPC, 1], f32)
        bqb_sb = const.tile([DPC, 1], f32)
        bv_sb = const.tile([DPC, 1], f32)
        ident = const.tile([128, 128], bf16)
        wq_sb = const.tile([128, KT, DPC], bf16)
        wk_sb = const.tile([128, KT, DPC], bf16)
        wv_sb = const.tile([128, KT, DPC], bf16)
        nc.sync.dma_start(rb_sb[:], rb[:])
        nc.sync.dma_start(bqa_sb[:], bqa[:])
        nc.sync.dma_start(bqb_sb[:], bqb[:])
        nc.sync.dma_start(bv_sb[:], bv[:])
        for w_sb, w in ((wq_sb, wq), (wk_sb, wk), (wv_sb, wv)):
            nc.sync.dma_start(w_sb[:],
                                w.rearrange("(kt p) d -> p kt d", p=128))
        masks.make_identity(nc, ident[:])

        # per-(batch, head) augmented activations: qhat/khat [128, S] bf16.
        # hl=0 layout: [q*s ; q+bqs] / [k ; segrep]
        # hl=1 layout flipped: [q+bqs ; q*s] / [segrep ; k]  (keeps every
        # PSUM->SBUF copy lane-aligned)
        qhs, khs, vsbs = [], [], []
        for b in range(B):
            qhs.append([res.tile([128, S], bf16, tag=f"qh{b}{hl}",
                                 name=f"qh{b}{hl}") for hl in range(2)])
            khs.append([res.tile([128, S], bf16, tag=f"kh{b}{hl}",
                                 name=f"kh{b}{hl}") for hl in range(2)])
            v = res.tile([128, SKT * 130], bf16, tag=f"vsb{b}",
                         name=f"vsb{b}")
            nc.vector.memset(v[:], 1.0)   # preset ones cols
            vsbs.append(v)

        def p1_half(b, half):
            """Projections for one half (1024 tokens) of batch b."""
            if True:
                ci = 2 * b + half
                cs = slice(ci * CHUNK, (ci + 1) * CHUNK)
                ls = slice(half * CHUNK, (half + 1) * CHUNK)
                xts = []
                for kt in range(KT):
                    xt = xt_pool.tile([128, CHUNK], bf16, tag="xt")
                    nc.sync.dma_start(xt[:], xb[kt * 128:(kt + 1) * 128, cs])
                    xts.append(xt)
                nc.sync.dma_start(khs[b][0][64:128, ls], srt[64:128, cs])
                nc.sync.dma_start(khs[b][1][0:64, ls], srt[0:64, cs])

                def proj(psum_tile, w_sb):
                    for kt in range(KT):
                        for nn in range(CHUNK // 512):
                            nc.tensor.matmul(
                                psum_tile[:, nn * 512:(nn + 1) * 512],
                                w_sb[:, kt, :],
                                xts[kt][:, nn * 512:(nn + 1) * 512],
                                start=(kt == 0), stop=(kt == KT - 1))

                # one Q pass: psum parts 0:64 = q_h0, 64:128 = q_h1 (plain).
                # scaled halves in-lane; plain+bqs halves bounce through DRAM
                # to reach the opposite partition range.
                qp = big_psum.tile([128, CHUNK], f32, tag="big", name="qp")
                proj(qp, wq_sb)
                # scaled halves on ACT (idle during projections):
                # out = in*s + bq*s  (bqa holds bq*s)
                mult = mybir.AluOpType.mult
                add = mybir.AluOpType.add
                nc.vector.tensor_scalar(qhs[b][0][0:64, ls], qp[0:64, :],
                                        SCALE, bqa_sb[0:64, 0:1],
                                        op0=mult, op1=add)
                nc.vector.tensor_scalar(qhs[b][1][64:128, ls], qp[64:128, :],
                                        SCALE, bqa_sb[64:128, 0:1],
                                        op0=mult, op1=add)
                qs = qstage_pool.tile([128, CHUNK], bf16, tag="qstage")
                nc.vector.tensor_scalar_add(qs[:], qp[:], bqb_sb[:, 0:1])
                nc.sync.dma_start(qbounce[0, ci], qs[0:64, :])
                nc.sync.dma_start(qbounce[1, ci], qs[64:128, :])
                nc.sync.dma_start(qhs[b][0][64:128, ls], qbounce[0, ci])
                nc.sync.dma_start(qhs[b][1][0:64, ls], qbounce[1, ci])

                kp = big_psum.tile([128, CHUNK], f32, tag="big")
                proj(kp, wk_sb)
                nc.vector.tensor_copy(khs[b][0][0:64, ls], kp[0:64, :])
                nc.vector.tensor_copy(khs[b][1][64:128, ls], kp[64:128, :])

                vp = big_psum.tile([128, CHUNK], f32, tag="big")
                proj(vp, wv_sb)
                vt = vt_pool.tile([128, CHUNK], bf16, tag="vt")
                nc.vector.tensor_scalar_add(vt[:], vp[:], bv_sb[:, 0:1])
                for tt in range(CHUNK // 128):
                    gt = half * (CHUNK // 128) + tt
                    vtp = small_psum.tile([128, 128], bf16, tag="small")
                    nc.tensor.transpose(
                        vtp[:], vt[:, tt * 128:(tt + 1) * 128], ident[:])
                    nc.vector.tensor_copy(
                        vsbs[b][:, gt * 130:(gt + 1) * 130]
                        .rearrange("p (h x) -> p h x", h=2)[:, :, 0:64],
                        vtp[:].rearrange("p (h d) -> p h d", h=2))

        def scores_iter(b, hl, qh, kt):
            """scores^T (K=128 augmented) for key tile kt -> exp -> pt."""
            sp = big_psum.tile([128, QBLK], f32, tag="big")
            ksl = khs[b][hl][:, kt * 128:(kt + 1) * 128]
            for nn in range(QBLK // 512):
                qsl = qhs[b][hl][:, qh * QBLK + nn * 512:
                                 qh * QBLK + (nn + 1) * 512]
                nc.tensor.matmul(sp[:, nn * 512:(nn + 1) * 512], ksl, qsl,
                                 start=True, stop=True)
            pt = pt_pool.tile([128, QBLK], bf16, tag="pt")
            col = hl * 64 + b * 16 + kt
            nc.scalar.activation(pt[:], sp[:], Exp,
                                 bias=rb_sb[:, col:col + 1], scale=1.0)
            return pt

        def pv_iter(b, hl, ctxp, pt, kt):
            """ctx^T += [V|1].T @ P^T for key tile kt (V stationary)."""
            vb = kt * 130 + hl * 65
            for nn in range(QBLK // 512):
                nc.tensor.matmul(ctxp[:, nn * 512:(nn + 1) * 512],
                                 vsbs[b][:, vb:vb + 65],
                                 pt[:, nn * 512:(nn + 1) * 512],
                                 start=(kt == 0), stop=(kt == SKT - 1))

        def norm_iter(hl, qh, ctxp, stage):
            """Transpose ctx^T back per query tile; divide by denominator."""
            pb = hl * 64
            ctxs = ctxs_pool.tile([65, QBLK], bf16, tag="ctxs")
            nc.vector.tensor_copy(ctxs[:, 0:QBLK // 2], ctxp[:, 0:QBLK // 2])
            nc.vector.tensor_copy(ctxs[:, QBLK // 2:], ctxp[:, QBLK // 2:])
            for qt in range(NQT):
                ctp = small_psum.tile([128, 65], bf16, tag="small")
                nc.tensor.transpose(ctp[:], ctxs[:, qt * 128:(qt + 1) * 128],
                                    ident[0:65, 0:65])
                gq = qh * NQT + qt
                rcp = rcp_pool.tile([128, 1], f32, tag="rcp")
                nc.vector.reciprocal(rcp[:], ctp[:, 64:65])
                nc.vector.tensor_scalar_mul(
                    stage[:, gq * 128 + pb:gq * 128 + pb + 64],
                    ctp[:, 0:64], rcp[:, 0:1])

        p1_half(0, 0)
        p1_half(0, 1)

        stages = {}

        def get_stage(b):
            if b not in stages:
                stages[b] = stage_pool.tile([128, 16 * 128], f32,
                                            tag="stage", name=f"stage{b}")
            return stages[b]

        def flush_out(b):
            nc.sync.dma_start(
                outd[b * S:(b + 1) * S, :]
                .rearrange("(gq q) hd -> q gq hd", q=128),
                get_stage(b)[:].rearrange("q (gq hd) -> q gq hd", hd=DPC))

        seq = [(b, hl, qh) for b in range(B) for hl in range(HPC)
               for qh in range(QH)]
        prev = None   # (b, hl, qh, pts)
        for (b, hl, qh) in seq:
            pts = []
            if prev is not None:
                pctxp = ctx_psum.tile([65, QBLK], f32, tag="ctx")
            for kt in range(SKT):
                pts.append(scores_iter(b, hl, qh, kt))
                if prev is not None:
                    pv_iter(prev[0], prev[1], pctxp, prev[3][kt], kt)
            if prev is not None:
                norm_iter(prev[1], prev[2], pctxp, get_stage(prev[0]))
                if prev[0] != b:
                    flush_out(prev[0])
            prev = (b, hl, qh, pts)
            # inject next batch's projections into this batch's stream
            if b + 1 < B:
                if (hl, qh) == (0, 1):
                    p1_half(b + 1, 0)
                elif (hl, qh) == (1, 0):
                    p1_half(b + 1, 1)
        # drain the final iteration's PV
        pctxp = ctx_psum.tile([65, QBLK], f32, tag="ctx")
        for kt in range(SKT):
            pv_iter(prev[0], prev[1], pctxp, prev[3][kt], kt)
        norm_iter(prev[1], prev[2], pctxp, get_stage(prev[0]))
        flush_out(prev[0])

    nc.compile()
    return nc


def get_program():
    if "nc" not in _cache:
        _cache["nc"] = _build_program()
    return _cache["nc"]


def make_in_maps(hidden_states, attention_mask, seg_ids, Wq, bq, Wk, Wv, bv,
                 seg_table, b_q_s):
    """Host-side shard + layout prep. Cheap (weights/bias reshapes, one bf16
    cast of X, 2-row segment gather); all O(T*S) math stays on device."""
    bf = ml_dtypes.bfloat16
    X = np.asarray(hidden_states, np.float32).reshape(T, HID)
    xb = np.ascontiguousarray(X.astype(bf).T)
    m = np.asarray(seg_ids).reshape(T).astype(np.int64)
    mask = np.asarray(attention_mask, np.float32).reshape(B, S)
    st = np.asarray(seg_table, np.float32)              # [2, HID]
    bqs = np.asarray(b_q_s, np.float32).reshape(NH, DH)
    Wq = np.asarray(Wq, np.float32)
    Wk = np.asarray(Wk, np.float32)
    Wv = np.asarray(Wv, np.float32)
    bq = np.asarray(bq, np.float32)
    bv = np.asarray(bv, np.float32)

    # mask-only per-key bias, same layout for both heads of a core:
    # rb[key, hl*64 + b*16 + kt] = mask[b, kt*128+key]
    rb_half = mask.reshape(B, 16, 128).transpose(2, 0, 1).reshape(128, 64)
    rb_c = np.ascontiguousarray(
        np.concatenate([rb_half, rb_half], axis=1).astype(np.float32))

    in_maps = []
    for c in range(N_CORES):
        h0, h1 = c * HPC, c * HPC + 1
        s0, s1 = slice(h0 * DH, (h0 + 1) * DH), slice(h1 * DH, (h1 + 1) * DH)
        # one plain Q weight slice; bias vectors: bqa = bq (scaled path
        # applies (q+bq)*s), bqb = bq + b_q_s (plain path)
        bqa_c = np.concatenate([bq[s0], bq[s1]]) * SCALE
        bqb_c = np.concatenate([bq[s0] + bqs[h0], bq[s1] + bqs[h1]])
        # segrep^T halves: [0:64]=head1, [64:128]=head0
        srt_c = np.empty((128, T), np.float32)
        srt_c[0:64, :] = st[np.ix_(m, range(s1.start, s1.stop))].T
        srt_c[64:128, :] = st[np.ix_(m, range(s0.start, s0.stop))].T
        sl = slice(c * DPC, (c + 1) * DPC)
        in_maps.append({
            "xb": xb,
            "wq": np.ascontiguousarray(Wq[sl, :].T).astype(bf),
            "wk": np.ascontiguousarray(Wk[sl, :].T).astype(bf),
            "wv": np.ascontiguousarray(Wv[sl, :].T).astype(bf),
            "srt": srt_c.astype(bf),
            "rb": rb_c,
            "bqa": np.ascontiguousarray(bqa_c.reshape(DPC, 1)),
            "bqb": np.ascontiguousarray(bqb_c.reshape(DPC, 1)),
            "bv": np.ascontiguousarray(bv[sl].reshape(DPC, 1)),
        })
    return in_maps


def assemble_output(results):
    return np.concatenate(
        [np.asarray(r["out"], np.float32).reshape(B, S, DPC) for r in results],
        axis=2)


def kernel(hidden_states, attention_mask, seg_ids, Wq, bq, Wk, Wv, bv,
           seg_table, b_q_s):
    from concourse.bass_utils import run_bass_kernel_spmd
    nc = get_program()
    in_maps = make_in_maps(hidden_states, attention_mask, seg_ids, Wq, bq,
                           Wk, Wv, bv, seg_table, b_q_s)
    res = run_bass_kernel_spmd(nc, in_maps, list(range(N_CORES)))
    return assemble_output(res.results)


if __name__ == "__main__":
    get_program()
    print("program built + compiled ok")


# revision 28
# speedup vs baseline: 1.0061x; 1.0061x over previous
"""BertSelfAttention (with segment-embedding score bias) on 8 trn2 NeuronCores.

Math implemented (reference semantics):
    q = X @ Wq.T + bq ; k = X @ Wk.T ; v = X @ Wv.T + bv      (per head h)
    scores = (q*s) @ k.T + (q + b_q_s) @ segrep.T + mask ;  s = 1/sqrt(DH)
    out = softmax(scores) @ v

Formulation: per head, augmented 128-deep contractions
    qhat = [q*s ; q + b_q_s]   (dims 0:64 scaled, 64:128 plain+bias)
    khat = [k   ; segrep     ] (segrep = seg_table[seg_ids] slice, host-prep)
    scores = qhat . khat  (exactly includes the segment term); mask is a
    per-key bias fused into the exp() activation. The K=128 contraction keeps
    the PE array fully occupied (half-height matmuls starve the activity
    monitor and the PE clock throttles to 1.2 GHz).
    Softmax denominator = ones-column appended to V in the PV matmul
    (ctx^T accumulated with V stationary, then transposed back per 128-query
    tile and scaled by the reciprocal denominator).

Sharding: tensor-parallel over heads; core c owns heads 2c, 2c+1.
Each core reads the full tokens, computes its head-slice and its slice of
the output; host concatenates along the hidden dim. No collectives.

Schedule: batches processed end-to-end; attention software-pipelined so PV
of iteration g-1 interleaves with scores of iteration g (PE densely busy
while ACT exp()s run concurrently).
"""

import os
import sys

for _p in ("/opt/trn_rl_repo", "/root/.axon_site/_ro/trn_rl_repo"):
    if os.path.isdir(_p) and _p not in sys.path:
        sys.path.append(_p)

import numpy as np
import ml_dtypes

B, S, NH, DH = 4, 2048, 16, 64
HID = NH * DH          # 1024
T = B * S              # 8192
N_CORES = 8
HPC = NH // N_CORES    # heads per core = 2
DPC = HPC * DH         # out dims per core = 128
SCALE = 1.0 / 8.0      # 1/sqrt(DH)
KT = HID // 128        # 8 contraction tiles
CHUNK = 1024           # token chunk for projections
SKT = S // 128         # 16 key tiles per sequence
QH = 2                 # query halves per sequence
QBLK = S // QH         # 1024
NQT = QBLK // 128      # 8 query tiles per half

_cache = {}


def _build_program():
    import concourse.bacc as bacc
    import concourse.tile as tile
    from concourse import masks, mybir
    from contextlib import ExitStack

    bf16 = mybir.dt.bfloat16
    f32 = mybir.dt.float32
    Exp = mybir.ActivationFunctionType.Exp
    Ident = mybir.ActivationFunctionType.Identity

    nc = bacc.Bacc("TRN2", target_bir_lowering=False, debug=False,
                   num_devices=N_CORES)
    xb = nc.dram_tensor("xb", [HID, T], bf16, kind="ExternalInput")
    wq = nc.dram_tensor("wq", [HID, DPC], bf16, kind="ExternalInput")
    wk = nc.dram_tensor("wk", [HID, DPC], bf16, kind="ExternalInput")
    wv = nc.dram_tensor("wv", [HID, DPC], bf16, kind="ExternalInput")
    srt = nc.dram_tensor("srt", [128, T], bf16, kind="ExternalInput")
    rb = nc.dram_tensor("rb", [128, 128], f32, kind="ExternalInput")
    bqa = nc.dram_tensor("bqa", [DPC, 1], f32, kind="ExternalInput")
    bqb = nc.dram_tensor("bqb", [DPC, 1], f32, kind="ExternalInput")
    bv = nc.dram_tensor("bv", [DPC, 1], f32, kind="ExternalInput")
    outd = nc.dram_tensor("out", [T, DPC], f32, kind="ExternalOutput")
    # cross-partition bounce for the plain-q half of qhat
    qbounce = nc.dram_tensor("qbounce", [2, T // CHUNK, 64, CHUNK], bf16)

    with tile.TileContext(nc) as tc, ExitStack() as octx:
        const = octx.enter_context(tc.tile_pool(name="const", bufs=1))
        res = octx.enter_context(tc.tile_pool(name="res", bufs=1))
        xt_pool = octx.enter_context(tc.tile_pool(name="xt", bufs=20))
        vt_pool = octx.enter_context(tc.tile_pool(name="vt", bufs=2))
        pt_pool = octx.enter_context(tc.tile_pool(name="pt", bufs=18))
        ctxs_pool = octx.enter_context(tc.tile_pool(name="ctxs", bufs=2))
        qstage_pool = octx.enter_context(tc.tile_pool(name="qstage", bufs=2))
        stage_pool = octx.enter_context(tc.tile_pool(name="stage", bufs=2))
        rcp_pool = octx.enter_context(tc.tile_pool(name="rcp", bufs=8))
        big_psum = octx.enter_context(
            tc.tile_pool(name="bigp", bufs=2, space="PSUM"))
        ctx_psum = octx.enter_context(
            tc.tile_pool(name="ctxp", bufs=1, space="PSUM"))
        small_psum = octx.enter_context(
            tc.tile_pool(name="smallp", bufs=2, space="PSUM"))

        # constants
        rb_sb = const.tile([128, 128], f32)
        bqa_sb = const.tile([DPC, 1], f32)
        bqb_sb = const.tile([DPC, 1], f32)
        bv_sb = const.tile([DPC, 1], f32)
        ident = const.tile([128, 128], bf16)
        wq_sb = const.tile([128, KT, DPC], bf16)
        wk_sb = const.tile([128, KT, DPC], bf16)
        wv_sb = const.tile([128, KT, DPC], bf16)
        nc.sync.dma_start(rb_sb[:], rb[:])
        nc.sync.dma_start(bqa_sb[:], bqa[:])
        nc.sync.dma_start(bqb_sb[:], bqb[:])
        nc.sync.dma_start(bv_sb[:], bv[:])
        for w_sb, w in ((wq_sb, wq), (wk_sb, wk), (wv_sb, wv)):
            nc.sync.dma_start(w_sb[:],
                                w.rearrange("(kt p) d -> p kt d", p=128))
        masks.make_identity(nc, ident[:])

        # per-(batch, head) augmented activations: qhat/khat [128, S] bf16.
        # hl=0 layout: [q*s ; q+bqs] / [k ; segrep]
        # hl=1 layout flipped: [q+bqs ; q*s] / [segrep ; k]  (keeps every
        # PSUM->SBUF copy lane-aligned)
        qhs, khs, vsbs = [], [], []
        for b in range(B):
            qhs.append([res.tile([128, S], bf16, tag=f"qh{b}{hl}",
                                 name=f"qh{b}{hl}") for hl in range(2)])
            khs.append([res.tile([128, S], bf16, tag=f"kh{b}{hl}",
                                 name=f"kh{b}{hl}") for hl in range(2)])
            v = res.tile([128, SKT * 130], bf16, tag=f"vsb{b}",
                         name=f"vsb{b}")
            nc.vector.memset(v[:], 1.0)   # preset ones cols
            vsbs.append(v)

        def p1_half(b, half):
            """Projections for one half (1024 tokens) of batch b."""
            if True:
                ci = 2 * b + half
                cs = slice(ci * CHUNK, (ci + 1) * CHUNK)
                ls = slice(half * CHUNK, (half + 1) * CHUNK)
                xts = []
                for kt in range(KT):
                    xt = xt_pool.tile([128, CHUNK], bf16, tag="xt")
                    nc.sync.dma_start(xt[:], xb[kt * 128:(kt + 1) * 128, cs])
                    xts.append(xt)
                nc.sync.dma_start(khs[b][0][64:128, ls], srt[64:128, cs])
                nc.sync.dma_start(khs[b][1][0:64, ls], srt[0:64, cs])

                def proj(psum_tile, w_sb):
                    for kt in range(KT):
                        for nn in range(CHUNK // 512):
                            nc.tensor.matmul(
                                psum_tile[:, nn * 512:(nn + 1) * 512],
                                w_sb[:, kt, :],
                                xts[kt][:, nn * 512:(nn + 1) * 512],
                                start=(kt == 0), stop=(kt == KT - 1))

                # one Q pass: psum parts 0:64 = q_h0, 64:128 = q_h1 (plain).
                # scaled halves in-lane; plain+bqs halves bounce through DRAM
                # to reach the opposite partition range.
                qp = big_psum.tile([128, CHUNK], f32, tag="big", name="qp")
                proj(qp, wq_sb)
                # scaled halves on ACT (idle during projections):
                # out = in*s + bq*s  (bqa holds bq*s)
                nc.scalar.activation(qhs[b][0][0:64, ls], qp[0:64, :], Ident,
                                     bias=bqa_sb[0:64, 0:1], scale=SCALE)
                nc.scalar.activation(qhs[b][1][64:128, ls], qp[64:128, :],
                                     Ident, bias=bqa_sb[64:128, 0:1],
                                     scale=SCALE)
                qs = qstage_pool.tile([128, CHUNK], bf16, tag="qstage")
                nc.vector.tensor_scalar_add(qs[:], qp[:], bqb_sb[:, 0:1])
                nc.sync.dma_start(qbounce[0, ci], qs[0:64, :])
                nc.sync.dma_start(qbounce[1, ci], qs[64:128, :])
                nc.sync.dma_start(qhs[b][0][64:128, ls], qbounce[0, ci])
                nc.sync.dma_start(qhs[b][1][0:64, ls], qbounce[1, ci])

                kp = big_psum.tile([128, CHUNK], f32, tag="big")
                proj(kp, wk_sb)
                nc.vector.tensor_copy(khs[b][0][0:64, ls], kp[0:64, :])
                nc.vector.tensor_copy(khs[b][1][64:128, ls], kp[64:128, :])

                vp = big_psum.tile([128, CHUNK], f32, tag="big")
                proj(vp, wv_sb)
                vt = vt_pool.tile([128, CHUNK], bf16, tag="vt")
                nc.vector.tensor_scalar_add(vt[:], vp[:], bv_sb[:, 0:1])
                for tt in range(CHUNK // 128):
                    gt = half * (CHUNK // 128) + tt
                    vtp = small_psum.tile([128, 128], bf16, tag="small")
                    nc.tensor.transpose(
                        vtp[:], vt[:, tt * 128:(tt + 1) * 128], ident[:])
                    nc.vector.tensor_copy(
                        vsbs[b][:, gt * 130:(gt + 1) * 130]
                        .rearrange("p (h x) -> p h x", h=2)[:, :, 0:64],
                        vtp[:].rearrange("p (h d) -> p h d", h=2))

        def scores_iter(b, hl, qh, kt):
            """scores^T (K=128 augmented) for key tile kt -> exp -> pt."""
            sp = big_psum.tile([128, QBLK], f32, tag="big")
            ksl = khs[b][hl][:, kt * 128:(kt + 1) * 128]
            for nn in range(QBLK // 512):
                qsl = qhs[b][hl][:, qh * QBLK + nn * 512:
                                 qh * QBLK + (nn + 1) * 512]
                nc.tensor.matmul(sp[:, nn * 512:(nn + 1) * 512], ksl, qsl,
                                 start=True, stop=True)
            pt = pt_pool.tile([128, QBLK], bf16, tag="pt")
            col = hl * 64 + b * 16 + kt
            nc.scalar.activation(pt[:], sp[:], Exp,
                                 bias=rb_sb[:, col:col + 1], scale=1.0)
            return pt

        def pv_iter(b, hl, ctxp, pt, kt):
            """ctx^T += [V|1].T @ P^T for key tile kt (V stationary)."""
            vb = kt * 130 + hl * 65
            for nn in range(QBLK // 512):
                nc.tensor.matmul(ctxp[:, nn * 512:(nn + 1) * 512],
                                 vsbs[b][:, vb:vb + 65],
                                 pt[:, nn * 512:(nn + 1) * 512],
                                 start=(kt == 0), stop=(kt == SKT - 1))

        def norm_iter(hl, qh, ctxp, stage):
            """Transpose ctx^T back per query tile; divide by denominator."""
            pb = hl * 64
            ctxs = ctxs_pool.tile([65, QBLK], bf16, tag="ctxs")
            nc.vector.tensor_copy(ctxs[:, 0:QBLK // 2], ctxp[:, 0:QBLK // 2])
            nc.vector.tensor_copy(ctxs[:, QBLK // 2:], ctxp[:, QBLK // 2:])
            for qt in range(NQT):
                ctp = small_psum.tile([128, 65], bf16, tag="small")
                nc.tensor.transpose(ctp[:], ctxs[:, qt * 128:(qt + 1) * 128],
                                    ident[0:65, 0:65])
                gq = qh * NQT + qt
                rcp = rcp_pool.tile([128, 1], f32, tag="rcp")
                nc.vector.reciprocal(rcp[:], ctp[:, 64:65])
                nc.vector.tensor_scalar_mul(
                    stage[:, gq * 128 + pb:gq * 128 + pb + 64],
                    ctp[:, 0:64], rcp[:, 0:1])

        p1_half(0, 0)
        p1_half(0, 1)

        stages = {}

        def get_stage(b):
            if b not in stages:
                stages[b] = stage_pool.tile([128, 16 * 128], f32,
                                            tag="stage", name=f"stage{b}")
            return stages[b]

        def flush_out(b):
            nc.sync.dma_start(
                outd[b * S:(b + 1) * S, :]
                .rearrange("(gq q) hd -> q gq hd", q=128),
                get_stage(b)[:].rearrange("q (gq hd) -> q gq hd", hd=DPC))

        seq = [(b, hl, qh) for b in range(B) for hl in range(HPC)
               for qh in range(QH)]
        prev = None   # (b, hl, qh, pts)
        for (b, hl, qh) in seq:
            pts = []
            if prev is not None:
                pctxp = ctx_psum.tile([65, QBLK], f32, tag="ctx")
            for kt in range(SKT):
                pts.append(scores_iter(b, hl, qh, kt))
                if prev is not None:
                    pv_iter(prev[0], prev[1], pctxp, prev[3][kt], kt)
            if prev is not None:
                norm_iter(prev[1], prev[2], pctxp, get_stage(prev[0]))
                if prev[0] != b:
                    flush_out(prev[0])
            prev = (b, hl, qh, pts)
            # inject next batch's projections into this batch's stream
            if b + 1 < B:
                if (hl, qh) == (0, 1):
                    p1_half(b + 1, 0)
                elif (hl, qh) == (1, 0):
                    p1_half(b + 1, 1)
        # drain the final iteration's PV
        pctxp = ctx_psum.tile([65, QBLK], f32, tag="ctx")
        for kt in range(SKT):
            pv_iter(prev[0], prev[1], pctxp, prev[3][kt], kt)
        norm_iter(prev[1], prev[2], pctxp, get_stage(prev[0]))
        flush_out(prev[0])

    nc.compile()
    return nc


def get_program():
    if "nc" not in _cache:
        _cache["nc"] = _build_program()
    return _cache["nc"]


def make_in_maps(hidden_states, attention_mask, seg_ids, Wq, bq, Wk, Wv, bv,
                 seg_table, b_q_s):
    """Host-side shard + layout prep. Cheap (weights/bias reshapes, one bf16
    cast of X, 2-row segment gather); all O(T*S) math stays on device."""
    bf = ml_dtypes.bfloat16
    X = np.asarray(hidden_states, np.float32).reshape(T, HID)
    xb = np.ascontiguousarray(X.astype(bf).T)
    m = np.asarray(seg_ids).reshape(T).astype(np.int64)
    mask = np.asarray(attention_mask, np.float32).reshape(B, S)
    st = np.asarray(seg_table, np.float32)              # [2, HID]
    bqs = np.asarray(b_q_s, np.float32).reshape(NH, DH)
    Wq = np.asarray(Wq, np.float32)
    Wk = np.asarray(Wk, np.float32)
    Wv = np.asarray(Wv, np.float32)
    bq = np.asarray(bq, np.float32)
    bv = np.asarray(bv, np.float32)

    # mask-only per-key bias, same layout for both heads of a core:
    # rb[key, hl*64 + b*16 + kt] = mask[b, kt*128+key]
    rb_half = mask.reshape(B, 16, 128).transpose(2, 0, 1).reshape(128, 64)
    rb_c = np.ascontiguousarray(
        np.concatenate([rb_half, rb_half], axis=1).astype(np.float32))

    in_maps = []
    for c in range(N_CORES):
        h0, h1 = c * HPC, c * HPC + 1
        s0, s1 = slice(h0 * DH, (h0 + 1) * DH), slice(h1 * DH, (h1 + 1) * DH)
        # one plain Q weight slice; bias vectors: bqa = bq (scaled path
        # applies (q+bq)*s), bqb = bq + b_q_s (plain path)
        bqa_c = np.concatenate([bq[s0], bq[s1]]) * SCALE
        bqb_c = np.concatenate([bq[s0] + bqs[h0], bq[s1] + bqs[h1]])
        # segrep^T halves: [0:64]=head1, [64:128]=head0
        srt_c = np.empty((128, T), np.float32)
        srt_c[0:64, :] = st[np.ix_(m, range(s1.start, s1.stop))].T
        srt_c[64:128, :] = st[np.ix_(m, range(s0.start, s0.stop))].T
        sl = slice(c * DPC, (c + 1) * DPC)
        in_maps.append({
            "xb": xb,
            "wq": np.ascontiguousarray(Wq[sl, :].T).astype(bf),
            "wk": np.ascontiguousarray(Wk[sl, :].T).astype(bf),
            "wv": np.ascontiguousarray(Wv[sl, :].T).astype(bf),
            "srt": srt_c.astype(bf),
            "rb": rb_c,
            "bqa": np.ascontiguousarray(bqa_c.reshape(DPC, 1)),
            "bqb": np.ascontiguousarray(bqb_c.reshape(DPC, 1)),
            "bv": np.ascontiguousarray(bv[sl].reshape(DPC, 1)),
        })
    return in_maps


def assemble_output(results):
    return np.concatenate(
        [np.asarray(r["out"], np.float32).reshape(B, S, DPC) for r in results],
        axis=2)


def kernel(hidden_states, attention_mask, seg_ids, Wq, bq, Wk, Wv, bv,
           seg_table, b_q_s):
    from concourse.bass_utils import run_bass_kernel_spmd
    nc = get_program()
    in_maps = make_in_maps(hidden_states, attention_mask, seg_ids, Wq, bq,
                           Wk, Wv, bv, seg_table, b_q_s)
    res = run_bass_kernel_spmd(nc, in_maps, list(range(N_CORES)))
    return assemble_output(res.results)


if __name__ == "__main__":
    get_program()
    print("program built + compiled ok")


# revision 29
# speedup vs baseline: 1.0124x; 1.0062x over previous
"""BertSelfAttention (with segment-embedding score bias) on 8 trn2 NeuronCores.

Math implemented (reference semantics):
    q = X @ Wq.T + bq ; k = X @ Wk.T ; v = X @ Wv.T + bv      (per head h)
    scores = (q*s) @ k.T + (q + b_q_s) @ segrep.T + mask ;  s = 1/sqrt(DH)
    out = softmax(scores) @ v

Formulation: per head, augmented 128-deep contractions
    qhat = [q*s ; q + b_q_s]   (dims 0:64 scaled, 64:128 plain+bias)
    khat = [k   ; segrep     ] (segrep = seg_table[seg_ids] slice, host-prep)
    scores = qhat . khat  (exactly includes the segment term); mask is a
    per-key bias fused into the exp() activation. The K=128 contraction keeps
    the PE array fully occupied (half-height matmuls starve the activity
    monitor and the PE clock throttles to 1.2 GHz).
    Softmax denominator = ones-column appended to V in the PV matmul
    (ctx^T accumulated with V stationary, then transposed back per 128-query
    tile and scaled by the reciprocal denominator).

Sharding: tensor-parallel over heads; core c owns heads 2c, 2c+1.
Each core reads the full tokens, computes its head-slice and its slice of
the output; host concatenates along the hidden dim. No collectives.

Schedule: batches processed end-to-end; attention software-pipelined so PV
of iteration g-1 interleaves with scores of iteration g (PE densely busy
while ACT exp()s run concurrently).
"""

import os
import sys

for _p in ("/opt/trn_rl_repo", "/root/.axon_site/_ro/trn_rl_repo"):
    if os.path.isdir(_p) and _p not in sys.path:
        sys.path.append(_p)

import numpy as np
import ml_dtypes

B, S, NH, DH = 4, 2048, 16, 64
HID = NH * DH          # 1024
T = B * S              # 8192
N_CORES = 8
HPC = NH // N_CORES    # heads per core = 2
DPC = HPC * DH         # out dims per core = 128
SCALE = 1.0 / 8.0      # 1/sqrt(DH)
KT = HID // 128        # 8 contraction tiles
CHUNK = 1024           # token chunk for projections
SKT = S // 128         # 16 key tiles per sequence
QH = 2                 # query halves per sequence
QBLK = S // QH         # 1024
NQT = QBLK // 128      # 8 query tiles per half

_cache = {}


def _build_program():
    import concourse.bacc as bacc
    import concourse.tile as tile
    from concourse import masks, mybir
    from contextlib import ExitStack

    bf16 = mybir.dt.bfloat16
    f32 = mybir.dt.float32
    Exp = mybir.ActivationFunctionType.Exp
    Ident = mybir.ActivationFunctionType.Identity

    nc = bacc.Bacc("TRN2", target_bir_lowering=False, debug=False,
                   num_devices=N_CORES)
    xb = nc.dram_tensor("xb", [HID, T], bf16, kind="ExternalInput")
    wq = nc.dram_tensor("wq", [HID, DPC], bf16, kind="ExternalInput")
    wk = nc.dram_tensor("wk", [HID, DPC], bf16, kind="ExternalInput")
    wv = nc.dram_tensor("wv", [HID, DPC], bf16, kind="ExternalInput")
    srt = nc.dram_tensor("srt", [128, T], bf16, kind="ExternalInput")
    rb = nc.dram_tensor("rb", [128, 128], f32, kind="ExternalInput")
    bqa = nc.dram_tensor("bqa", [DPC, 1], f32, kind="ExternalInput")
    bqb = nc.dram_tensor("bqb", [DPC, 1], f32, kind="ExternalInput")
    bv = nc.dram_tensor("bv", [DPC, 1], f32, kind="ExternalInput")
    outd = nc.dram_tensor("out", [T, DPC], f32, kind="ExternalOutput")
    # cross-partition bounce for the plain-q half of qhat
    qbounce = nc.dram_tensor("qbounce", [2, T // CHUNK, 64, CHUNK], bf16)

    with tile.TileContext(nc) as tc, ExitStack() as octx:
        const = octx.enter_context(tc.tile_pool(name="const", bufs=1))
        res = octx.enter_context(tc.tile_pool(name="res", bufs=1))
        xt_pool = octx.enter_context(tc.tile_pool(name="xt", bufs=20))
        vt_pool = octx.enter_context(tc.tile_pool(name="vt", bufs=2))
        pt_pool = octx.enter_context(tc.tile_pool(name="pt", bufs=18))
        ctxs_pool = octx.enter_context(tc.tile_pool(name="ctxs", bufs=2))
        qstage_pool = octx.enter_context(tc.tile_pool(name="qstage", bufs=2))
        stage_pool = octx.enter_context(tc.tile_pool(name="stage", bufs=2))
        rcp_pool = octx.enter_context(tc.tile_pool(name="rcp", bufs=8))
        big_psum = octx.enter_context(
            tc.tile_pool(name="bigp", bufs=2, space="PSUM"))
        ctx_psum = octx.enter_context(
            tc.tile_pool(name="ctxp", bufs=1, space="PSUM"))
        small_psum = octx.enter_context(
            tc.tile_pool(name="smallp", bufs=2, space="PSUM"))

        # constants
        rb_sb = const.tile([128, 128], f32)
        bqa_sb = const.tile([DPC, 1], f32)
        bqb_sb = const.tile([DPC, 1], f32)
        bv_sb = const.tile([DPC, 1], f32)
        ident = const.tile([128, 128], bf16)
        wq_sb = const.tile([128, KT, DPC], bf16)
        wk_sb = const.tile([128, KT, DPC], bf16)
        wv_sb = const.tile([128, KT, DPC], bf16)
        nc.sync.dma_start(rb_sb[:], rb[:])
        nc.sync.dma_start(bqa_sb[:], bqa[:])
        nc.sync.dma_start(bqb_sb[:], bqb[:])
        nc.sync.dma_start(bv_sb[:], bv[:])
        for w_sb, w in ((wq_sb, wq), (wk_sb, wk), (wv_sb, wv)):
            nc.sync.dma_start(w_sb[:],
                                w.rearrange("(kt p) d -> p kt d", p=128))
        masks.make_identity(nc, ident[:])
        # PE warmup: ~3.5us of dense matmuls un-throttles the clock gate
        # while the first input DMAs are still in flight.
        wup = big_psum.tile([128, 512], f32, tag="big", name="wup")
        for _ in range(18):
            nc.tensor.matmul(wup[:, 0:128], ident[:], ident[:],
                             start=True, stop=True)

        # per-(batch, head) augmented activations: qhat/khat [128, S] bf16.
        # hl=0 layout: [q*s ; q+bqs] / [k ; segrep]
        # hl=1 layout flipped: [q+bqs ; q*s] / [segrep ; k]  (keeps every
        # PSUM->SBUF copy lane-aligned)
        qhs, khs, vsbs = [], [], []
        for b in range(B):
            qhs.append([res.tile([128, S], bf16, tag=f"qh{b}{hl}",
                                 name=f"qh{b}{hl}") for hl in range(2)])
            khs.append([res.tile([128, S], bf16, tag=f"kh{b}{hl}",
                                 name=f"kh{b}{hl}") for hl in range(2)])
            v = res.tile([128, SKT * 130], bf16, tag=f"vsb{b}",
                         name=f"vsb{b}")
            nc.vector.memset(v[:], 1.0)   # preset ones cols
            vsbs.append(v)

        def p1_half(b, half):
            """Projections for one half (1024 tokens) of batch b."""
            if True:
                ci = 2 * b + half
                cs = slice(ci * CHUNK, (ci + 1) * CHUNK)
                ls = slice(half * CHUNK, (half + 1) * CHUNK)
                xts = []
                for kt in range(KT):
                    xt = xt_pool.tile([128, CHUNK], bf16, tag="xt")
                    nc.sync.dma_start(xt[:], xb[kt * 128:(kt + 1) * 128, cs])
                    xts.append(xt)
                nc.sync.dma_start(khs[b][0][64:128, ls], srt[64:128, cs])
                nc.sync.dma_start(khs[b][1][0:64, ls], srt[0:64, cs])

                def proj(psum_tile, w_sb):
                    for kt in range(KT):
                        for nn in range(CHUNK // 512):
                            nc.tensor.matmul(
                                psum_tile[:, nn * 512:(nn + 1) * 512],
                                w_sb[:, kt, :],
                                xts[kt][:, nn * 512:(nn + 1) * 512],
                                start=(kt == 0), stop=(kt == KT - 1))

                # one Q pass: psum parts 0:64 = q_h0, 64:128 = q_h1 (plain).
                # scaled halves in-lane; plain+bqs halves bounce through DRAM
                # to reach the opposite partition range.
                qp = big_psum.tile([128, CHUNK], f32, tag="big", name="qp")
                proj(qp, wq_sb)
                # scaled halves on ACT (idle during projections):
                # out = in*s + bq*s  (bqa holds bq*s)
                nc.scalar.activation(qhs[b][0][0:64, ls], qp[0:64, :], Ident,
                                     bias=bqa_sb[0:64, 0:1], scale=SCALE)
                nc.scalar.activation(qhs[b][1][64:128, ls], qp[64:128, :],
                                     Ident, bias=bqa_sb[64:128, 0:1],
                                     scale=SCALE)
                qs = qstage_pool.tile([128, CHUNK], bf16, tag="qstage")
                nc.vector.tensor_scalar_add(qs[:], qp[:], bqb_sb[:, 0:1])
                nc.sync.dma_start(qbounce[0, ci], qs[0:64, :])
                nc.sync.dma_start(qbounce[1, ci], qs[64:128, :])
                nc.sync.dma_start(qhs[b][0][64:128, ls], qbounce[0, ci])
                nc.sync.dma_start(qhs[b][1][0:64, ls], qbounce[1, ci])

                kp = big_psum.tile([128, CHUNK], f32, tag="big")
                proj(kp, wk_sb)
                nc.vector.tensor_copy(khs[b][0][0:64, ls], kp[0:64, :])
                nc.vector.tensor_copy(khs[b][1][64:128, ls], kp[64:128, :])

                vp = big_psum.tile([128, CHUNK], f32, tag="big")
                proj(vp, wv_sb)
                vt = vt_pool.tile([128, CHUNK], bf16, tag="vt")
                nc.vector.tensor_scalar_add(vt[:], vp[:], bv_sb[:, 0:1])
                for tt in range(CHUNK // 128):
                    gt = half * (CHUNK // 128) + tt
                    vtp = small_psum.tile([128, 128], bf16, tag="small")
                    nc.tensor.transpose(
                        vtp[:], vt[:, tt * 128:(tt + 1) * 128], ident[:])
                    nc.vector.tensor_copy(
                        vsbs[b][:, gt * 130:(gt + 1) * 130]
                        .rearrange("p (h x) -> p h x", h=2)[:, :, 0:64],
                        vtp[:].rearrange("p (h d) -> p h d", h=2))

        def scores_iter(b, hl, qh, kt):
            """scores^T (K=128 augmented) for key tile kt -> exp -> pt."""
            sp = big_psum.tile([128, QBLK], f32, tag="big")
            ksl = khs[b][hl][:, kt * 128:(kt + 1) * 128]
            for nn in range(QBLK // 512):
                qsl = qhs[b][hl][:, qh * QBLK + nn * 512:
                                 qh * QBLK + (nn + 1) * 512]
                nc.tensor.matmul(sp[:, nn * 512:(nn + 1) * 512], ksl, qsl,
                                 start=True, stop=True)
            pt = pt_pool.tile([128, QBLK], bf16, tag="pt")
            col = hl * 64 + b * 16 + kt
            nc.scalar.activation(pt[:], sp[:], Exp,
                                 bias=rb_sb[:, col:col + 1], scale=1.0)
            return pt

        def pv_iter(b, hl, ctxp, pt, kt):
            """ctx^T += [V|1].T @ P^T for key tile kt (V stationary)."""
            vb = kt * 130 + hl * 65
            for nn in range(QBLK // 512):
                nc.tensor.matmul(ctxp[:, nn * 512:(nn + 1) * 512],
                                 vsbs[b][:, vb:vb + 65],
                                 pt[:, nn * 512:(nn + 1) * 512],
                                 start=(kt == 0), stop=(kt == SKT - 1))

        def norm_iter(hl, qh, ctxp, stage):
            """Transpose ctx^T back per query tile; divide by denominator."""
            pb = hl * 64
            ctxs = ctxs_pool.tile([65, QBLK], bf16, tag="ctxs")
            nc.vector.tensor_copy(ctxs[:, 0:QBLK // 2], ctxp[:, 0:QBLK // 2])
            nc.vector.tensor_copy(ctxs[:, QBLK // 2:], ctxp[:, QBLK // 2:])
            for qt in range(NQT):
                ctp = small_psum.tile([128, 65], bf16, tag="small")
                nc.tensor.transpose(ctp[:], ctxs[:, qt * 128:(qt + 1) * 128],
                                    ident[0:65, 0:65])
                gq = qh * NQT + qt
                rcp = rcp_pool.tile([128, 1], f32, tag="rcp")
                nc.vector.reciprocal(rcp[:], ctp[:, 64:65])
                nc.vector.tensor_scalar_mul(
                    stage[:, gq * 128 + pb:gq * 128 + pb + 64],
                    ctp[:, 0:64], rcp[:, 0:1])

        p1_half(0, 0)
        p1_half(0, 1)

        stages = {}

        def get_stage(b):
            if b not in stages:
                stages[b] = stage_pool.tile([128, 16 * 128], f32,
                                            tag="stage", name=f"stage{b}")
            return stages[b]

        def flush_out(b):
            nc.sync.dma_start(
                outd[b * S:(b + 1) * S, :]
                .rearrange("(gq q) hd -> q gq hd", q=128),
                get_stage(b)[:].rearrange("q (gq hd) -> q gq hd", hd=DPC))

        seq = [(b, hl, qh) for b in range(B) for hl in range(HPC)
               for qh in range(QH)]
        prev = None   # (b, hl, qh, pts)
        for (b, hl, qh) in seq:
            pts = []
            if prev is not None:
                pctxp = ctx_psum.tile([65, QBLK], f32, tag="ctx")
            for kt in range(SKT):
                pts.append(scores_iter(b, hl, qh, kt))
                if prev is not None:
                    pv_iter(prev[0], prev[1], pctxp, prev[3][kt], kt)
            if prev is not None:
                norm_iter(prev[1], prev[2], pctxp, get_stage(prev[0]))
                if prev[0] != b:
                    flush_out(prev[0])
            prev = (b, hl, qh, pts)
            # inject next batch's projections into this batch's stream
            if b + 1 < B:
                if (hl, qh) == (0, 1):
                    p1_half(b + 1, 0)
                elif (hl, qh) == (1, 0):
                    p1_half(b + 1, 1)
        # drain the final iteration's PV
        pctxp = ctx_psum.tile([65, QBLK], f32, tag="ctx")
        for kt in range(SKT):
            pv_iter(prev[0], prev[1], pctxp, prev[3][kt], kt)
        norm_iter(prev[1], prev[2], pctxp, get_stage(prev[0]))
        flush_out(prev[0])

    nc.compile()
    return nc


def get_program():
    if "nc" not in _cache:
        _cache["nc"] = _build_program()
    return _cache["nc"]


def make_in_maps(hidden_states, attention_mask, seg_ids, Wq, bq, Wk, Wv, bv,
                 seg_table, b_q_s):
    """Host-side shard + layout prep. Cheap (weights/bias reshapes, one bf16
    cast of X, 2-row segment gather); all O(T*S) math stays on device."""
    bf = ml_dtypes.bfloat16
    X = np.asarray(hidden_states, np.float32).reshape(T, HID)
    xb = np.ascontiguousarray(X.astype(bf).T)
    m = np.asarray(seg_ids).reshape(T).astype(np.int64)
    mask = np.asarray(attention_mask, np.float32).reshape(B, S)
    st = np.asarray(seg_table, np.float32)              # [2, HID]
    bqs = np.asarray(b_q_s, np.float32).reshape(NH, DH)
    Wq = np.asarray(Wq, np.float32)
    Wk = np.asarray(Wk, np.float32)
    Wv = np.asarray(Wv, np.float32)
    bq = np.asarray(bq, np.float32)
    bv = np.asarray(bv, np.float32)

    # mask-only per-key bias, same layout for both heads of a core:
    # rb[key, hl*64 + b*16 + kt] = mask[b, kt*128+key]
    rb_half = mask.reshape(B, 16, 128).transpose(2, 0, 1).reshape(128, 64)
    rb_c = np.ascontiguousarray(
        np.concatenate([rb_half, rb_half], axis=1).astype(np.float32))

    in_maps = []
    for c in range(N_CORES):
        h0, h1 = c * HPC, c * HPC + 1
        s0, s1 = slice(h0 * DH, (h0 + 1) * DH), slice(h1 * DH, (h1 + 1) * DH)
        # one plain Q weight slice; bias vectors: bqa = bq (scaled path
        # applies (q+bq)*s), bqb = bq + b_q_s (plain path)
        bqa_c = np.concatenate([bq[s0], bq[s1]]) * SCALE
        bqb_c = np.concatenate([bq[s0] + bqs[h0], bq[s1] + bqs[h1]])
        # segrep^T halves: [0:64]=head1, [64:128]=head0
        srt_c = np.empty((128, T), np.float32)
        srt_c[0:64, :] = st[np.ix_(m, range(s1.start, s1.stop))].T
        srt_c[64:128, :] = st[np.ix_(m, range(s0.start, s0.stop))].T
        sl = slice(c * DPC, (c + 1) * DPC)
        in_maps.append({
            "xb": xb,
            "wq": np.ascontiguousarray(Wq[sl, :].T).astype(bf),
            "wk": np.ascontiguousarray(Wk[sl, :].T).astype(bf),
            "wv": np.ascontiguousarray(Wv[sl, :].T).astype(bf),
            "srt": srt_c.astype(bf),
            "rb": rb_c,
            "bqa": np.ascontiguousarray(bqa_c.reshape(DPC, 1)),
            "bqb": np.ascontiguousarray(bqb_c.reshape(DPC, 1)),
            "bv": np.ascontiguousarray(bv[sl].reshape(DPC, 1)),
        })
    return in_maps


def assemble_output(results):
    return np.concatenate(
        [np.asarray(r["out"], np.float32).reshape(B, S, DPC) for r in results],
        axis=2)


def kernel(hidden_states, attention_mask, seg_ids, Wq, bq, Wk, Wv, bv,
           seg_table, b_q_s):
    from concourse.bass_utils import run_bass_kernel_spmd
    nc = get_program()
    in_maps = make_in_maps(hidden_states, attention_mask, seg_ids, Wq, bq,
                           Wk, Wv, bv, seg_table, b_q_s)
    res = run_bass_kernel_spmd(nc, in_maps, list(range(N_CORES)))
    return assemble_output(res.results)


if __name__ == "__main__":
    get_program()
    print("program built + compiled ok")
